# revision 1
# baseline (speedup 1.0000x reference)
"""Multi-head self-attention kernel for Trainium2 (8 NeuronCores), v2.

Problem: B=2, S=2048, D=1024, H=16 heads of hd=64.
Sharding: core c handles batch b=c//4 and head-group hg=c%4 (4 heads each).

Design (vs the f32r baseline):
  - scores via fp8e4(e4m3) DoubleRow matmuls: q,k quantized to fp8 after the
    bf16 qkv projection; per head a [32, 2, *] layout packs hd=64 as
    32 partitions x 2 packed rows -> 0.5 cycles/row on the PE.
  - AV restructured: stationary = e^T chunk [128kj, 128qi] (full PE tile),
    moving = v [128, 65] (ones column -> denominator), output o[qi, 4, 65] in
    natural layout; epilogue is per-partition reciprocal+scale.
  - exp split across ACT (exact Exp) and DVE (Schraudolph int16 bit trick
    into bf16); gpsimd cannot touch PSUM.
  - ctx -> ctx^T via DMA transpose (SBUF->SBUF bf16, xbar), spread across
    slots and hwdge queues.
  - global software pipeline: AV trails its exp by AV_LAG slots so PE
    instructions never stall the 4-deep wait queue.
"""

import sys

sys.path.insert(0, "/opt/trn_rl_repo")

import ml_dtypes
import numpy as np

import concourse.bass as bass
import concourse.tile as tile
from concourse import bacc, mybir
from concourse.bass_utils import run_bass_kernel_spmd

B, S, D = 2, 2048, 1024
H, HD = 16, 64
HL = 4  # heads per core
P = 128
KC = D // P  # 8 contraction chunks over D
NQ = 4  # qi chunks of 512
NKJ = 16  # kj chunks of 128
F32 = mybir.dt.float32
BF16 = mybir.dt.bfloat16
FP8 = mybir.dt.float8e4
I16 = mybir.dt.int16

LOG2E = 1.4426950408889634
SCH_A = 16.0 * LOG2E  # folds the 1/8 score scale: s/8 * log2(e) * 128
SCH_B = 127.0 * 128.0 - 6.0  # bf16 exponent bias<<7, Schraudolph-centered

MULT = mybir.AluOpType.mult
ADD = mybir.AluOpType.add

import os

AV_LAG = int(os.environ.get("K_AV_LAG", "5"))
FUSED_NORM = os.environ.get("K_FUSED_NORM", "1") == "1"
PS_S_BUFS = int(os.environ.get("K_PS_S", "5"))
PS_AV_BUFS = int(os.environ.get("K_PS_AV", "1"))

# per-block exp engine pattern (A=ACT exact exp, D=DVE Schraudolph),
# ACT-leaning at the block tail so DVE is free for the epilogue recip/norm
EXP_PAT = os.environ.get("K_EXP_PAT", "ADADAADADADADADA")
TP_ENG = os.environ.get("K_TP_ENG", "sync")
COPY_MODE = os.environ.get("K_COPY", "dve")


def exp_on_act(idx):
    return EXP_PAT[idx % len(EXP_PAT)] == "A"


def build_program():
    nc = bacc.Bacc("TRN2", target_bir_lowering=False)

    xt_d = nc.dram_tensor("xt", [D, S], BF16, kind="ExternalInput")
    wqk_d = nc.dram_tensor("wqk", [D, 4 * P], BF16, kind="ExternalInput")
    bqk_d = nc.dram_tensor("bqk", [4 * P], F32, kind="ExternalInput")
    wv_d = nc.dram_tensor("wv", [D, HL * HD], BF16, kind="ExternalInput")
    bv_d = nc.dram_tensor("bv", [HL * HD], BF16, kind="ExternalInput")
    wp_d = nc.dram_tensor("wp", [HL * HD, D], BF16, kind="ExternalInput")
    out_d = nc.dram_tensor("out", [D, S], BF16, kind="ExternalOutput")

    out_v = out_d.rearrange("(mo p) s -> p mo s", p=P)  # [128, 8, 2048]

    with tile.TileContext(nc) as tc:
        with (
            tc.tile_pool(name="const", bufs=1) as const,
            tc.tile_pool(name="xp", bufs=1) as xp,
            tc.tile_pool(name="pexp", bufs=16) as pexp,
            tc.tile_pool(name="prc", bufs=6) as prc,
            tc.tile_pool(name="pctx", bufs=3) as pctx,
            tc.tile_pool(name="pctxT", bufs=3) as pctxT,
            tc.tile_pool(name="pout", bufs=8) as pout,
            tc.tile_pool(name="ps_s", bufs=PS_S_BUFS, space="PSUM") as ps_s,
            tc.tile_pool(name="ps_mm", bufs=2, space="PSUM") as ps_mm,
            tc.tile_pool(name="ps_av", bufs=PS_AV_BUFS, space="PSUM") as ps_av,
        ):
            # ---- input DMAs: wqk m2 chunk + xt n0 first (gate first matmul) ----
            wqk_sb = const.tile([P, KC, 4 * P], BF16)
            wqk_v = wqk_d.rearrange("(kc p) m -> p kc m", p=P)
            nc.sync.dma_start(wqk_sb[:, :, 2 * P : 3 * P], wqk_v[:, :, 2 * P : 3 * P])
            xt_sb = [
                [
                    xp.tile([P, 512], BF16, tag=f"xt{kc}n{n}", name=f"xt{kc}n{n}")
                    for n in range(NQ)
                ]
                for kc in range(KC)
            ]
            xt_v = xt_d.rearrange("(kc p) n -> p kc n", p=P)
            for kc in range(KC):
                (nc.scalar if kc % 2 == 0 else nc.sync).dma_start(
                    xt_sb[kc][0][:], xt_v[:, kc, 0:512]
                )
            nc.sync.dma_start(wqk_sb[:, :, 3 * P :], wqk_v[:, :, 3 * P :])
            bqk_sb = const.tile([P, 4], F32)
            nc.sync.dma_start(bqk_sb[:], bqk_d.rearrange("(m p) -> p m", p=P))
            nc.sync.dma_start(wqk_sb[:, :, 0:P], wqk_v[:, :, 0:P])
            nc.sync.dma_start(wqk_sb[:, :, P : 2 * P], wqk_v[:, :, P : 2 * P])
            wv_sb = const.tile([P, KC, 256], BF16)
            nc.sync.dma_start(wv_sb[:], wv_d.rearrange("(kc p) m -> p kc m", p=P))
            bv_sb = const.tile([1, 256], BF16)
            nc.sync.dma_start(bv_sb[:], bv_d[:].unsqueeze(0))
            # later qi chunks of x: optionally one wide DMA per n (fewer
            # instructions on the serial DMA path); tiles stay per-kc views
            XT_BATCH = os.environ.get("K_XT_BATCH", "1") == "1"
            if XT_BATCH:
                xtn_sb = {
                    n: xp.tile([P, KC, 512], BF16, tag=f"xtn{n}", name=f"xtn{n}")
                    for n in range(1, NQ)
                }
            for n in range(1, NQ):
                if XT_BATCH:
                    for kc in range(KC):
                        xt_sb[kc][n] = xtn_sb[n][:, kc, :]
                    nc.sync.dma_start(
                        xtn_sb[n][:], xt_v[:, :, n * 512 : (n + 1) * 512]
                    )
                else:
                    for kc in range(KC):
                        nc.sync.dma_start(
                            xt_sb[kc][n][:], xt_v[:, kc, n * 512 : (n + 1) * 512]
                        )
                if n == 1:
                    wp_sb = const.tile([P, 2, D], BF16)
                    nc.sync.dma_start(wp_sb[:], wp_d.rearrange("(c p) m -> p c m", p=P))

            # ---- persistent SBUF tiles ----
            # q8/k8: [128 = 4 heads x 32 hd-low, 2 = hd-high, 2048 tokens] fp8
            q8_sb = const.tile([P, 2, S], FP8)
            k8_sb = const.tile([P, 2, S], FP8)
            # v: per 128-token chunk, [128 tok, 4 heads, 66] bf16 (col 64 = 1)
            v_sb = [
                const.tile([P, HL, 66], BF16, tag=f"v{s}", name=f"v{s}")
                for s in range(NKJ)
            ]
            ones_sb = const.tile([1, P], BF16)
            nc.gpsimd.memset(ones_sb[:], 1.0)
            from concourse import masks
            ident_sb = const.tile([P, P], BF16)
            masks.make_identity(nc, ident_sb[:])
            for s in range(NKJ):
                nc.gpsimd.memset(v_sb[s][:, :, 64:65], 1.0)

            # ---- qkv helpers (matmul part and bias/convert part split) ----
            def qk_mm(m, n):
                pst = ps_mm.tile([P, 512], F32, tag="mm", name="pst")
                for kc in range(KC):
                    nc.tensor.matmul(
                        pst[:],
                        wqk_sb[:, kc, m * P : (m + 1) * P],
                        xt_sb[kc][n][:],
                        start=(kc == 0),
                        stop=(kc == KC - 1),
                    )
                return pst

            QKBIAS_2STEP = os.environ.get("K_QKBIAS_2STEP", "0") == "1"

            def qk_bias(m, n, pst):
                dst = (q8_sb if m < 2 else k8_sb)[:, m % 2, n * 512 : (n + 1) * 512]
                if QKBIAS_2STEP:
                    tmp = pexp.tile([P, 512], BF16, tag="qb", bufs=2, name="tmp")
                    nc.scalar.activation(
                        tmp[:],
                        pst[:],
                        mybir.ActivationFunctionType.Identity,
                        bias=bqk_sb[:, m : m + 1],
                    )
                    nc.vector.tensor_copy(dst, tmp[:])
                else:
                    nc.scalar.activation(
                        dst,
                        pst[:],
                        mybir.ActivationFunctionType.Identity,
                        bias=bqk_sb[:, m : m + 1],
                    )

            def v_mm(s):
                # bias folded in via a rank-1 ones x bv matmul
                pst = ps_mm.tile([P, 256], F32, tag="mm", name="pst")
                for kc in range(KC):
                    nc.tensor.matmul(
                        pst[:],
                        xt_sb[kc][s // 4][:, (s % 4) * P : (s % 4 + 1) * P],
                        wv_sb[:, kc, :],
                        start=(kc == 0),
                        stop=False,
                    )
                nc.tensor.matmul(
                    pst[:], ones_sb[:], bv_sb[:], start=False, stop=True
                )
                return pst

            def v_copy(s, pst):
                nc.scalar.activation(
                    v_sb[s][:, :, 0:64],
                    pst[:].rearrange("p (h c) -> p h c", h=HL),
                    mybir.ActivationFunctionType.Copy,
                )

            # ---- attention pieces ----
            exbank = {}  # (qc, h, kj) -> prefetched e-tile

            def pf_filler(qc, h, kj):
                def f():
                    ps = emit_scores(h, qc, kj)
                    ex = pexp.tile([P, 512], BF16, tag="pf", bufs=16, name="pfex")
                    if exp_on_act(kj):
                        nc.scalar.activation(
                            ex[:], ps[:], mybir.ActivationFunctionType.Exp,
                            scale=0.125,
                        )
                    else:
                        nc.vector.tensor_scalar(
                            ex[:].bitcast(I16), ps[:], SCH_A, SCH_B, MULT, ADD
                        )
                    exbank[(qc, h, kj)] = ex
                return f

            ctx_tiles = {}  # pair -> current ctx tile [128, 4, 128] bf16
            ctxT_tiles = {}  # qc -> ctxT tile [128, 2, 512] bf16
            po_tiles = {}  # (h, qc) -> psum [128, 4, 66]
            exp_idx = [0]

            def emit_scores(h, qc, kj):
                ps = ps_s.tile([P, 512], F32, tag="sc", name="ps")
                nc.tensor.matmul(
                    ps[:],
                    k8_sb[32 * h : 32 * h + 32, :, kj * P : (kj + 1) * P],
                    q8_sb[32 * h : 32 * h + 32, :, qc * 512 : (qc + 1) * 512],
                    start=True,
                    stop=True,
                    perf_mode=mybir.MatmulPerfMode.DoubleRow,
                    tile_position=(32 * h, 0),
                )
                return ps

            def emit_exp(ps, kj):
                ex = pexp.tile([P, 512], BF16, tag="ex", name="ex")
                if exp_on_act(kj):
                    nc.scalar.activation(
                        ex[:], ps[:], mybir.ActivationFunctionType.Exp, scale=0.125
                    )
                else:
                    nc.vector.tensor_scalar(
                        ex[:].bitcast(I16), ps[:], SCH_A, SCH_B, MULT, ADD
                    )
                return ex

            def emit_av(h, qc, kj, ex):
                if kj == 0:
                    po_tiles[(h, qc)] = ps_av.tile(
                        [P, 4, 66], F32, tag="av", name="po"
                    )
                po = po_tiles[(h, qc)]
                for qs in range(4):
                    # start=True zeroes the whole PSUM bank, so only the
                    # first matmul of the block may carry it
                    nc.tensor.matmul(
                        po[:, qs, 0:65],
                        ex[:, qs * P : (qs + 1) * P],
                        v_sb[kj][:, h, 0:65],
                        start=(kj == 0 and qs == 0),
                        stop=(kj == NKJ - 1),
                        skip_group_check=True,
                    )

            def epilogue(h, qc):
                po = po_tiles.pop((h, qc))
                pair = h // 2
                if h % 2 == 0:
                    ctx_tiles[pair] = pctx.tile(
                        [P, 4, P], BF16, tag=f"ctx{pair}", name=f"ctx{pair}"
                    )
                ct = ctx_tiles[pair]
                rc = prc.tile([P, 4, 1], F32, tag="rc", name="rc")
                nc.vector.reciprocal(rc[:], po[:, :, 64:65])
                pb = 64 * (h % 2)
                if FUSED_NORM:
                    nc.vector.tensor_mul(
                        ct[:, :, pb : pb + 64],
                        po[:, :, 0:64],
                        rc[:].broadcast_to([P, 4, 64]),
                    )
                else:
                    for qs in range(4):
                        nc.vector.tensor_scalar(
                            ct[:, qs, pb : pb + 64],
                            po[:, qs, 0:64],
                            rc[:, qs, :],
                            None,
                            MULT,
                        )

            def emit_transpose(qc, pair):
                # one xbar block-transpose: ctx [qi, (qs d)] -> ctxT [d, (qs qi)]
                (nc.sync if TP_ENG == "sync" else nc.scalar).dma_start_transpose(
                    ctxT_tiles[qc][:, pair, :].rearrange("p (qs d) -> p qs d", qs=4),
                    ctx_tiles[pair][:, :, :],
                )

            def proj_mm(qc, mo, pool=None, c_order=(0, 1)):
                pp = (pool or ps_mm).tile([P, 512], F32,
                                          tag="mm" if pool is None else "sc",
                                          name="pp")
                for i, c in enumerate(c_order):
                    nc.tensor.matmul(
                        pp[:],
                        wp_sb[:, c, mo * P : (mo + 1) * P],
                        ctxT_tiles[qc][:, c, :],
                        start=(i == 0),
                        stop=(i == 1),
                    )
                return pp

            OUT_PAIR = os.environ.get("K_OUT_PAIR", "1") == "1"
            OUT_DMA = os.environ.get("K_OUT_DMA", "gpsimd")
            ot_pend = {}

            def proj_out(qc, mo, pp):
                if qc == NQ - 1:
                    # tail: latency matters; copies alternate engines, DMAs
                    # ride the idle SP queue
                    eng = nc.sync
                    use_act = mo % 2 == 0
                else:
                    eng = {"gpsimd": nc.gpsimd, "sync": nc.sync,
                           "scalar": nc.scalar}.get(OUT_DMA, nc.sync)
                    use_act = (COPY_MODE == "act") or (
                        COPY_MODE == "alt" and mo % 2 == 0)
                if not OUT_PAIR:
                    ot = pout.tile([P, 512], BF16, tag="ot", name="ot")
                    if use_act:
                        nc.scalar.activation(
                            ot[:], pp[:], mybir.ActivationFunctionType.Copy
                        )
                    else:
                        nc.vector.tensor_copy(ot[:], pp[:])
                    eng.dma_start(
                        out_v[:, mo, qc * 512 : (qc + 1) * 512], ot[:]
                    )
                    return
                # stage two mo chunks per [128, 2, 512] tile -> one out DMA
                if mo % 2 == 0:
                    ot_pend[qc] = pout.tile([P, 2, 512], BF16, tag="ot2", name="ot")
                ot = ot_pend[qc]
                if use_act:
                    nc.scalar.activation(
                        ot[:, mo % 2, :], pp[:], mybir.ActivationFunctionType.Copy
                    )
                else:
                    nc.vector.tensor_copy(ot[:, mo % 2, :], pp[:])
                if mo % 2 == 1:
                    eng.dma_start(
                        out_v[:, mo - 1 : mo + 1, qc * 512 : (qc + 1) * 512],
                        ot_pend.pop(qc)[:],
                    )

            # ---- priming: q/k n0 chunks + v 0..3 ----
            for m in (2, 3, 0, 1):
                pst = qk_mm(m, 0)
                qk_bias(m, 0, pst)
            for s in range(4):
                pst = v_mm(s)
                v_copy(s, pst)

            # ---- filler schedule ----
            # fill[(qc, h)][slot] = list of closures
            def qk_filler(m, n):
                st = {}

                def mm():
                    st["ps"] = qk_mm(m, n)

                def bias():
                    qk_bias(m, n, st.pop("ps"))

                return mm, bias

            def v_filler(s):
                st = {}

                def mm():
                    st["ps"] = v_mm(s)

                def cp():
                    v_copy(s, st.pop("ps"))

                return mm, cp

            def proj_filler(qc, mo):
                st = {}

                def mm():
                    st["ps"] = proj_mm(qc, mo)

                def cp():
                    proj_out(qc, mo, st.pop("ps"))

                return mm, cp

            # execution order of (qc, h) blocks; last qc runs pair1 heads
            # first so its ctx^T transpose hides inside the loop
            block_list = []
            for qc in range(NQ):
                heads = (2, 3, 0, 1) if qc == NQ - 1 else (0, 1, 2, 3)
                for h in heads:
                    block_list.append((qc, h))
            blk_idx = {bh: i for i, bh in enumerate(block_list)}

            fill = {}
            for qc in range(NQ):
                for h in range(HL):
                    fill[(qc, h)] = [[] for _ in range(NKJ)]

            # qc0 h0: JIT k chunks (n ready before kj=4n) and v chunks
            # (v chunks wait for the xt n1..n3 DMAs, so they sit late)
            slots_h0 = {
                0: [qk_filler(2, 1)], 1: [qk_filler(3, 1)],
                4: [qk_filler(2, 2)], 5: [qk_filler(3, 2)],
                6: [v_filler(4)], 7: [v_filler(5)],
                8: [qk_filler(2, 3), v_filler(6)],
                9: [qk_filler(3, 3), v_filler(7)],
                10: [v_filler(8)], 11: [v_filler(9)],
                12: [v_filler(10)], 13: [v_filler(11)],
                14: [v_filler(12)], 15: [v_filler(13)],
            }
            for sl, fs in slots_h0.items():
                for mm, bias in fs:
                    fill[(0, 0)][sl].append(mm)
                    fill[(0, 0)][min(sl + 1, 15)].append(bias)
            # v14, v15 early in h1
            for i, s in enumerate((14, 15)):
                mm, cp = v_filler(s)
                fill[(0, 1)][2 * i].append(mm)
                fill[(0, 1)][2 * i + 1].append(cp)
            # q chunks for qc1 stay in qc0-h1; q chunks for qc2/qc3 move
            # into the otherwise-bare first blocks of qc1/qc2 (those are
            # exp-bound; qc0 is PE-bound, so this work was on the
            # critical path there)
            for i, m in enumerate((0, 1)):
                mm, bias = qk_filler(m, 1)
                fill[(0, 0)][2 + i].append(mm)
                fill[(0, 0)][3 + i].append(bias)
            for kj in range(NKJ):
                fill[(0, 0)][8 + kj // 2].append(pf_filler(1, 0, kj))
            QB = int(os.environ.get("K_QB", "3"))
            QS = int(os.environ.get("K_QS", "6"))
            for qc, n in ((1, 2), (2, 3)):
                for i, m in enumerate((0, 1)):
                    mm, bias = qk_filler(m, n)
                    fill[(qc, 0)][min(QB + QS * i, 15)].append(mm)
                    fill[(qc, 0)][min(QB + 2 + QS * i, 15)].append(bias)
            # qc >= 1: proj of qc-1 spread over blocks 1..3 of qc with wide
            # spacing (the ctx^T transpose DMA needs ~3us after the last
            # epilogue of qc-1; copies trail their matmuls by 2 slots)
            for qc in range(1, NQ):
                blocks_of_qc = [bh for bh in block_list if bh[0] == qc]
                mo = 0
                PB = int(os.environ.get("K_PROJ_BASE", "2"))
                PS = int(os.environ.get("K_PROJ_STRIDE", "5"))
                for bi, mos in ((1, 3), (2, 3), (3, 2)):
                    bh = blocks_of_qc[bi]
                    for i in range(mos):
                        mm, cp = proj_filler(qc - 1, mo)
                        fill[bh][min(PB + PS * i, 15)].append(mm)
                        fill[bh][min(PB + 2 + PS * i, 15)].append(cp)
                        mo += 1

            # transpose of (qc, pair) goes 1 block after its last head, at
            # slot 5 (right after the pair's last epilogue lands at slot 4)
            tp_sched = {}
            tail_tp = []
            for qc in range(NQ):
                for pair in range(2):
                    last = max(blk_idx[(qc, 2 * pair)], blk_idx[(qc, 2 * pair + 1)])
                    if last + 1 < len(block_list):
                        tp_sched.setdefault(block_list[last + 1], []).append(
                            (qc, pair)
                        )
                    else:
                        tail_tp.append((qc, pair))

            # ---- main slot loop ----
            pending = []  # (due_slot, order, fn)
            slot_no = [0]
            order_no = [0]

            def at_slot(lag, fn):
                pending.append((slot_no[0] + lag, order_no[0], fn))
                order_no[0] += 1

            def run_due():
                pending.sort(key=lambda e: (e[0], e[1]))
                while pending and pending[0][0] <= slot_no[0]:
                    pending.pop(0)[2]()

            last_bh = block_list[-1]
            for qc, h in block_list:
                if qc not in ctxT_tiles:
                    ctxT_tiles[qc] = pctxT.tile(
                        [P, 2, 512], BF16, tag="ctxT", name=f"ctxT{qc}"
                    )
                for qcp, pair in tp_sched.get((qc, h), []):
                    at_slot(int(os.environ.get("K_TP_LAG", "5")),
                            lambda a=qcp, b=pair: emit_transpose(a, b))
                for kj in range(NKJ):
                    run_due()
                    if (qc, h, kj) in exbank:
                        ex = exbank.pop((qc, h, kj))
                    else:
                        ps = emit_scores(h, qc, kj)
                        ex = emit_exp(ps, kj)
                    for f in fill[(qc, h)][kj]:
                        f()
                    lag = AV_LAG
                    if (qc, h) == last_bh and kj >= NKJ - AV_LAG:
                        lag = max(1, NKJ - kj)  # shorten the tail drain
                    at_slot(lag, lambda h=h, qc=qc, kj=kj, ex=ex:
                            emit_av(h, qc, kj, ex))
                    slot_no[0] += 1
                # MUST be emitted before the next block's first AV: pool WAR
                # protection is emission-ordered and av kj0 zeroes the bank
                at_slot(AV_LAG, lambda h=h, qc=qc: epilogue(h, qc))

            # ---- tail: drain pending avs, last transpose, last projection.
            # The last qc ran pair-1 heads first, so c=1 of its projection can
            # pre-start on freed score-pool banks while pair 0 finishes. ----
            pending.sort(key=lambda e: (e[0], e[1]))
            drained = 0
            tail_pps = {}
            while pending:
                pending.pop(0)[2]()
                drained += 1
                if drained == 2:
                    for mo in range(4):
                        tail_pps[mo] = proj_mm(
                            NQ - 1, mo, pool=ps_s, c_order=(1,)
                        )
            for qcp, pair in tail_tp:
                if os.environ.get("K_TAIL_PE_TP", "1") == "1":
                    for qs in range(4):
                        tp = ps_mm.tile([P, P], BF16, tag="mm", name="tp")
                        nc.tensor.transpose(
                            tp[:], ctx_tiles[pair][:, qs, :], ident_sb[:]
                        )
                        nc.vector.tensor_copy(
                            ctxT_tiles[qcp][:, pair, qs * P : (qs + 1) * P], tp[:]
                        )
                else:
                    emit_transpose(qcp, pair)
            for mo in range(8):
                if mo in tail_pps:
                    pp = tail_pps[mo]
                    nc.tensor.matmul(
                        pp[:],
                        wp_sb[:, 0, mo * P : (mo + 1) * P],
                        ctxT_tiles[NQ - 1][:, 0, :],
                        start=False,
                        stop=True,
                    )
                else:
                    pp = proj_mm(NQ - 1, mo)
                proj_out(NQ - 1, mo, pp)

    nc.compile()
    return nc


_NC = None


def _get_program():
    global _NC
    if _NC is None:
        _NC = build_program()
    return _NC


def _perm256():
    """Row order for a 256-row (4-head) q or k block: two 128-chunks
    (hd-high 0/1), partition p = 32*h + dlo -> row h*64 + dhi*32 + dlo."""
    idx = []
    for dhi in range(2):
        for h in range(4):
            for dlo in range(32):
                idx.append(h * 64 + dhi * 32 + dlo)
    return np.array(idx)


def prepare_inputs(x, w_qkv, b_qkv, w_proj):
    """Build the 8 per-core input maps from full inputs."""
    perm = _perm256()
    in_maps = []
    for c in range(8):
        b, hg = c // 4, c % 4
        sl = slice(hg * 256, (hg + 1) * 256)
        w_q, w_k, w_v = w_qkv[0:D][sl], w_qkv[D : 2 * D][sl], w_qkv[2 * D :][sl]
        b_q, b_k = b_qkv[0:D][sl], b_qkv[D : 2 * D][sl]
        wqk = np.vstack([w_q[perm], w_k[perm]])  # [512, 1024]
        bqk = np.concatenate([b_q[perm], b_k[perm]])
        in_maps.append(
            {
                "xt": np.ascontiguousarray(x[b].T).astype(ml_dtypes.bfloat16),
                "wqk": np.ascontiguousarray(wqk.T).astype(ml_dtypes.bfloat16),
                "bqk": np.ascontiguousarray(bqk.astype(np.float32)),
                "wv": np.ascontiguousarray(w_v.T).astype(ml_dtypes.bfloat16),
                "bv": np.ascontiguousarray(
                    b_qkv[2 * D :][sl].astype(ml_dtypes.bfloat16)
                ),
                "wp": np.ascontiguousarray(w_proj[:, sl].T).astype(ml_dtypes.bfloat16),
            }
        )
    return in_maps


def run(in_maps, **kwargs):
    nc = _get_program()
    last_err = None
    for _ in range(3):
        try:
            res = run_bass_kernel_spmd(nc, in_maps, core_ids=list(range(8)), **kwargs)
            res.results = [
                {k: np.array(v, dtype=np.float32) for k, v in r.items()}
                for r in res.results
            ]
            return res
        except Exception as e:  # transient NRT_EXEC_UNIT_UNRECOVERABLE etc.
            last_err = e
    raise last_err


def assemble(results, b_proj):
    out = np.empty((B, S, D), dtype=np.float32)
    for b in range(B):
        acc = results[4 * b]["out"].copy()
        for hg in range(1, 4):
            acc += results[4 * b + hg]["out"]
        out[b] = acc.T + b_proj
    return out


def kernel(x, w_qkv, b_qkv, w_proj, b_proj):
    x = np.asarray(x, dtype=np.float32)
    w_qkv = np.asarray(w_qkv, dtype=np.float32)
    b_qkv = np.asarray(b_qkv, dtype=np.float32)
    w_proj = np.asarray(w_proj, dtype=np.float32)
    b_proj = np.asarray(b_proj, dtype=np.float32)
    res = run(prepare_inputs(x, w_qkv, b_qkv, w_proj))
    return assemble(res.results, b_proj)



# revision 78
# speedup vs baseline: 1.1495x; 1.1495x over previous
"""Multi-head self-attention kernel for Trainium2 (8 NeuronCores), v3.

Problem: B=2, S=2048, D=1024, H=16 heads of hd=64.
Sharding: core c handles batch b=c//4 and head-group hg=c%4 (4 heads each).

v3 design (vs the v2 fp8-scores baseline; 151553 -> 131845 ns TimelineSim):
  - qkv projections in fp8 hi/lo: x = x8(e4m3) + xr(e5m2), w = w8 + wr;
    x@w ~= x8@w8 + xr@w8 + x8@wr as DoubleRow matmuls with 256-wide
    contraction -> 0.75x the bf16 PE cost at bf16-grade accuracy (no
    measurable rel-err change vs bf16 projections).
  - paired PSUM elementwise: scores, qk/out copies and v copies process
    two PSUM banks per instruction, amortizing the fixed access latency.
  - PSUM = one 3-deep pool of [128, 2, 512] pair tiles (6 banks) for every
    pair tile (scores / qk proj / out proj) + 2 scratch banks that serve
    the v JIT early and then the po accumulators, double-buffered across
    blocks; out-proj singles run in the window after each epilogue frees
    its scratch bank (qc2's last four go to the tail as ring pairs, with
    their copies forced onto ACT so the DVE transpose copies aren't
    queued behind them).
  - exp pairs split across ACT (exact Exp) and DVE (Schraudolph int16 bit
    trick into bf16) by a Bresenham balance pattern (60% ACT).
  - score+exp stream decoupled from the AV stream (SE_LEAD pairs ahead,
    AV trails AV_LAG pairs, deep SBUF ex pool) so the exp engines stay fed
    through the qkv JIT phase.
  - scheduling invariants: every consumer of a shared PSUM bank or JIT'd
    SBUF tile must be EMITTED before the next producer that reuses it --
    the pending queue orders epilogue -> proj -> next-block AV, the v/q/k
    readiness gates are copy_slot + 1 (fillers run after SE advances
    within a slot), and pair transposes are emitted directly inside the
    pair's second epilogue.
"""

import os
import sys

sys.path.insert(0, "/opt/trn_rl_repo")

import ml_dtypes
import numpy as np

import concourse.bass as bass
import concourse.tile as tile
from concourse import bacc, mybir
from concourse.bass_utils import run_bass_kernel_spmd

B, S, D = 2, 2048, 1024
H, HD = 16, 64
HL = 4  # heads per core
P = 128
KC = 4  # contraction chunks of 256 (DoubleRow packed [128, 2])
NQ = 4  # qi chunks of 512
NKJ = 16  # kj chunks of 128
NP = 8  # kj pairs per block
F32 = mybir.dt.float32
BF16 = mybir.dt.bfloat16
FP8 = mybir.dt.float8e4
FP85 = mybir.dt.float8e5
I16 = mybir.dt.int16

LOG2E = 1.4426950408889634
SCH_A = 16.0 * LOG2E  # folds the 1/8 score scale: s/8 * log2(e) * 128
SCH_B = 127.0 * 128.0 - 6.0  # bf16 exponent bias<<7, Schraudolph-centered

MULT = mybir.AluOpType.mult
ADD = mybir.AluOpType.add
DR = mybir.MatmulPerfMode.DoubleRow

AV_LAG = int(os.environ.get("K_AV_LAG", "8"))  # pairs
SE_LEAD = int(os.environ.get("K_SE_LEAD", "22"))  # max pairs score/exp ahead of slot
SE_PER_SLOT = int(os.environ.get("K_SE_PER_SLOT", "4"))
PEXP_BUFS = int(os.environ.get("K_PEXP", "28"))
# epilogue(b) MUST be emitted before AV(b+1, p0): pool WAR protection is
# emission-ordered and that AV's start=True zeroes the po bank. With lag 0
# it sorts between AV(b, p7) and AV(b+1, p0) by order number.
EPI_LAG = int(os.environ.get("K_EPI_LAG", "0"))
TP_LAG = int(os.environ.get("K_TP_LAG", "6"))
# exp engine pattern over global pair index: A=ACT exact, D=DVE Schraudolph.
# Default: Bresenham string with the ACT fraction that balances engine totals.
def _bresenham_pat(frac_a, n=128):
    acc, out = 0.0, []
    for _ in range(n):
        acc += frac_a
        if acc >= 1.0:
            out.append("A")
            acc -= 1.0
        else:
            out.append("D")
    return "".join(out)


_split = int(os.environ.get("K_EXP_SPLIT_AT", "40"))
EXP_PAT = os.environ.get("K_EXP_PAT", "") or (
    _bresenham_pat(float(os.environ.get("K_EXP_FRAC_A", "0.6")), _split)
    + _bresenham_pat(float(os.environ.get("K_EXP_FRAC_B", "0.6")),
                     128 - _split))
V_CP = os.environ.get("K_V_CP", "dve")
QK_CP = os.environ.get("K_QK_CP", "split")
OUT_CP = os.environ.get("K_OUT_CP", "alt")
OUT_DMA = os.environ.get("K_OUT_DMA", "sync")


def build_program():
    nc = bacc.Bacc("TRN2", target_bir_lowering=False)

    x8_d = nc.dram_tensor("x8", [P, KC, 2, S], FP8, kind="ExternalInput")
    xr5_d = nc.dram_tensor("xr5", [P, KC, 2, S], FP85, kind="ExternalInput")
    w8_d = nc.dram_tensor("w8", [P, KC, 2, 4 * P], FP8, kind="ExternalInput")
    wr5_d = nc.dram_tensor("wr5", [P, KC, 2, 4 * P], FP85, kind="ExternalInput")
    wv8_d = nc.dram_tensor("wv8", [P, KC, 2, 2 * P], FP8, kind="ExternalInput")
    wvr5_d = nc.dram_tensor("wvr5", [P, KC, 2, 2 * P], FP85, kind="ExternalInput")
    bqk_d = nc.dram_tensor("bqk", [P, 4], F32, kind="ExternalInput")
    bv_d = nc.dram_tensor("bv", [2 * P], BF16, kind="ExternalInput")
    wp_d = nc.dram_tensor("wp", [HL * HD, D], BF16, kind="ExternalInput")
    out_d = nc.dram_tensor("out", [D, S], BF16, kind="ExternalOutput")
    DEBUG = os.environ.get("K_DEBUG", "0") == "1"
    if DEBUG:
        dq8_d = nc.dram_tensor("dq8", [P, 2, S], FP8, kind="ExternalOutput")
        dk8_d = nc.dram_tensor("dk8", [P, 2, S], FP8, kind="ExternalOutput")
        dv_d = nc.dram_tensor("dv", [P, NKJ, HL, 66], BF16,
                              kind="ExternalOutput")
        dctxT_d = nc.dram_tensor("dctxT", [P, 2, 512], BF16,
                                 kind="ExternalOutput")


    out_v = out_d.rearrange("(mo p) s -> p mo s", p=P)  # [128, 8, 2048]

    with tile.TileContext(nc) as tc:
        with (
            tc.tile_pool(name="const", bufs=1) as const,
            tc.tile_pool(name="pexp", bufs=PEXP_BUFS) as pexp,
            tc.tile_pool(name="prc", bufs=4) as prc,
            tc.tile_pool(name="pctx", bufs=2) as pctx,
            tc.tile_pool(name="pctxT", bufs=2) as pctxT,
            tc.tile_pool(name="pout", bufs=6) as pout,
            tc.tile_pool(name="ps_s", bufs=3, space="PSUM") as ps_s,
            tc.tile_pool(name="ps_scr", bufs=1, space="PSUM") as ps_scr,
        ):
            # PSUM: one 3-deep pool of [128, 2, 512] pair tiles (6 banks)
            # holds every pair tile -- scores, qk proj, out proj -- so three
            # pairs are in flight; 2 scratch banks serve the v JIT early and
            # then the po accumulators, double-buffered across blocks.
            scr_sb = [ps_scr.tile([P, 512], F32, tag=f"scr{i}",
                                  name=f"scr{i}") for i in range(2)]

            def ring_pair():
                return ps_s.tile([P, 2, 512], F32, tag="sc", name="pr")
            # ---- persistent SBUF tiles ----
            x8_sb = const.tile([P, KC, 2, S], FP8)
            xr5_sb = const.tile([P, KC, 2, S], FP85)
            w8_sb = const.tile([P, KC, 2, 4 * P], FP8)
            wr5_sb = const.tile([P, KC, 2, 4 * P], FP85)
            wv8_sb = const.tile([P, KC, 2, 2 * P], FP8)
            wvr5_sb = const.tile([P, KC, 2, 2 * P], FP85)
            bqk_sb = const.tile([P, 4], F32)
            bv_sb = const.tile([1, 2 * P], BF16)
            wp_sb = const.tile([P, 2, D], BF16)
            q8_sb = const.tile([P, 2, S], FP8)
            k8_sb = const.tile([P, 2, S], FP8)
            v_all = const.tile([P, NKJ, HL, 66], BF16)
            ones_sb = const.tile([1, P], BF16)

            # ---- input DMAs: few, large transfers (each HWDGE issue costs
            # ~625ns serially); order gates the priming matmuls earliest ----
            nc.sync.dma_start(w8_sb[:], w8_d[:])
            nc.scalar.dma_start(x8_sb[:, :, :, 0:512], x8_d[:, :, :, 0:512])
            nc.sync.dma_start(bqk_sb[:], bqk_d[:])
            nc.sync.dma_start(wr5_sb[:], wr5_d[:])
            nc.scalar.dma_start(xr5_sb[:, :, :, 0:512], xr5_d[:, :, :, 0:512])
            nc.sync.dma_start(wv8_sb[:], wv8_d[:])
            nc.sync.dma_start(wvr5_sb[:], wvr5_d[:])
            nc.scalar.dma_start(bv_sb[:], bv_d[:].unsqueeze(0))
            for n in range(1, NQ):
                nc.sync.dma_start(
                    x8_sb[:, :, :, n * 512:(n + 1) * 512],
                    x8_d[:, :, :, n * 512:(n + 1) * 512])
                nc.scalar.dma_start(
                    xr5_sb[:, :, :, n * 512:(n + 1) * 512],
                    xr5_d[:, :, :, n * 512:(n + 1) * 512])
            # wp is not read until the first out-proj window (~slot 40);
            # emitting it last keeps the x chunk stream contiguous
            nc.sync.dma_start(
                wp_sb[:], wp_d.rearrange("(c p) m -> p c m", p=P))

            nc.gpsimd.memset(ones_sb[:], 1.0)
            from concourse import masks
            ident_sb = const.tile([P, P], BF16)
            masks.make_identity(nc, ident_sb[:])
            nc.gpsimd.memset(v_all[:, :, :, 64:65], 1.0)

            # ---- qkv projection (hi/lo fp8 DoubleRow) ----
            def qk_pair_mm(which, n):
                """One [128, 2, 512] PSUM pair: halves are the two m chunks
                (dhi 0/1) of q or k for token chunk n."""
                pst = ring_pair()
                ms = (0, 1) if which == "q" else (2, 3)
                for half, m in enumerate(ms):
                    terms = (
                        [(w8_sb, x8_sb)] * KC
                        + [(w8_sb, xr5_sb)] * KC
                        + [(wr5_sb, x8_sb)] * KC
                    )
                    for i, (wt, xt) in enumerate(terms):
                        kc = i % KC
                        nc.tensor.matmul(
                            pst[:, half, :],
                            wt[:, kc, :, m * P:(m + 1) * P],
                            xt[:, kc, :, n * 512:(n + 1) * 512],
                            start=(i == 0),
                            stop=(i == 3 * KC - 1),
                            perf_mode=DR,
                        )
                return pst

            def qk_copy(which, n, pst, half):
                m = ((0, 1) if which == "q" else (2, 3))[half]
                dst = (q8_sb if which == "q" else k8_sb)[
                    :, half, n * 512:(n + 1) * 512]
                use_dve = QK_CP == "dve" or (QK_CP == "split" and half == 1)
                if use_dve:
                    nc.vector.tensor_scalar(
                        dst, pst[:, half, :], bqk_sb[:, m:m + 1], None, ADD)
                else:
                    nc.scalar.activation(
                        dst, pst[:, half, :],
                        mybir.ActivationFunctionType.Identity,
                        bias=bqk_sb[:, m:m + 1],
                    )

            def v_pair_mm(sp):
                """v for token chunks 2sp, 2sp+1 -> one-bank [128, 2, 256]
                PSUM pair in a scratch bank. start=True only on the very
                first matmul (zeroes the whole shared bank); everything
                after accumulates."""
                pst = scr_sb[sp % 2].rearrange("p (a c) -> p a c", a=2)
                for half in (0, 1):
                    s = 2 * sp + half
                    n, t = s // 4, s % 4
                    terms = (
                        [(x8_sb, wv8_sb)] * KC
                        + [(xr5_sb, wv8_sb)] * KC
                        + [(x8_sb, wvr5_sb)] * KC
                    )
                    for i, (xt, wt) in enumerate(terms):
                        kc = i % KC
                        nc.tensor.matmul(
                            pst[:, half, :],
                            xt[:, kc, :, n * 512 + t * P:n * 512 + (t + 1) * P],
                            wt[:, kc, :, :],
                            start=(half == 0 and i == 0),
                            stop=False,
                            perf_mode=DR,
                            skip_group_check=(half == 1),
                        )
                    nc.tensor.matmul(
                        pst[:, half, :], ones_sb[:], bv_sb[:],
                        start=False, stop=True, skip_group_check=True)
                return pst

            def v_copy(sp, pst):
                src = pst[:].rearrange("p a (h c) -> p a h c", h=HL)
                dst = v_all[:, 2 * sp:2 * sp + 2, :, 0:64]
                if V_CP == "dve":
                    nc.vector.tensor_copy(dst, src)
                else:
                    nc.scalar.activation(
                        dst, src, mybir.ActivationFunctionType.Copy)

            # ---- attention ----
            def score_pair_mm(h, qc, p):
                ps = ring_pair()
                for half in (0, 1):
                    kj = 2 * p + half
                    nc.tensor.matmul(
                        ps[:, half, :],
                        k8_sb[32 * h:32 * h + 32, :, kj * P:(kj + 1) * P],
                        q8_sb[32 * h:32 * h + 32, :, qc * 512:(qc + 1) * 512],
                        start=True,
                        stop=True,
                        perf_mode=DR,
                        tile_position=(32 * h, 0),
                    )
                return ps

            def emit_exp(ps, gi):
                ex = pexp.tile([P, 2, 512], BF16, tag="ex", name="ex")
                if EXP_PAT[gi % len(EXP_PAT)] == "A":
                    nc.scalar.activation(
                        ex[:], ps[:], mybir.ActivationFunctionType.Exp,
                        scale=0.125)
                else:
                    nc.vector.tensor_scalar(
                        ex[:].bitcast(I16), ps[:], SCH_A, SCH_B, MULT, ADD)
                return ex

            po_tiles = {}
            po_flip = [0]

            def emit_av(h, qc, p, ex):
                if p == 0:
                    scr = scr_sb[po_flip[0] % 2]
                    po_flip[0] += 1
                    po_tiles[(h, qc)] = scr.rearrange(
                        "p (qs c) -> p qs c", qs=4)
                po = po_tiles[(h, qc)]
                for half in (0, 1):
                    kj = 2 * p + half
                    for qs in range(4):
                        nc.tensor.matmul(
                            po[:, qs, 0:65],
                            ex[:, half, qs * P:(qs + 1) * P],
                            v_all[:, kj, h, 0:65],
                            start=(p == 0 and half == 0 and qs == 0),
                            stop=(p == NP - 1 and half == 1),
                            skip_group_check=True,
                        )

            ctx_tiles = {}
            ctxT_tiles = {}

            epi_done = {}

            def epilogue(h, qc):
                po = po_tiles.pop((h, qc))
                pair = h // 2
                if (qc, pair) not in epi_done:
                    epi_done[(qc, pair)] = 0
                    ctx_tiles[pair] = pctx.tile(
                        [P, 4, P], BF16, tag=f"ctx{pair}", name=f"ctx{pair}")
                ct = ctx_tiles[pair]
                rc = prc.tile([P, 4, 1], F32, tag="rc", name="rc")
                nc.vector.reciprocal(rc[:], po[:, :, 64:65])
                pb = 64 * (h % 2)
                nc.vector.tensor_mul(
                    ct[:, :, pb:pb + 64],
                    po[:, :, 0:64],
                    rc[:].broadcast_to([P, 4, 64]),
                )
                epi_done[(qc, pair)] += 1
                if epi_done[(qc, pair)] == 2 and (qc, pair) not in tail_tp:
                    # emit the pair transpose right here: everything that
                    # reads ctxT[qc] is scheduled after this epilogue, so
                    # emission order is structurally safe
                    emit_transpose(qc, pair)

            def emit_transpose(qc, pair):
                nc.sync.dma_start_transpose(
                    ctxT_tiles[qc][:, pair, :].rearrange(
                        "p (qs d) -> p qs d", qs=4),
                    ctx_tiles[pair][:, :, :],
                )

            def proj_pair_mm(qc, mp, c_order=(0, 1)):
                pp = ring_pair()
                for half in (0, 1):
                    mo = 2 * mp + half
                    for i, c in enumerate(c_order):
                        nc.tensor.matmul(
                            pp[:, half, :],
                            wp_sb[:, c, mo * P:(mo + 1) * P],
                            ctxT_tiles[qc][:, c, :],
                            start=(i == 0),
                            stop=(i == len(c_order) - 1),
                        )
                return pp

            def proj_single_mm(qc, mo, scr):
                pp = scr[:].unsqueeze(1)[:, 0, :]
                for i, c in enumerate((0, 1)):
                    nc.tensor.matmul(
                        pp,
                        wp_sb[:, c, mo * P:(mo + 1) * P],
                        ctxT_tiles[qc][:, c, :],
                        start=(i == 0),
                        stop=(i == 1),
                    )
                return pp

            ot_pend = {}

            def proj_out_single(qc, mo, pp):
                if mo % 2 == 0:
                    ot_pend[qc] = pout.tile(
                        [P, 2, 512], BF16, tag="ot", name="ot")
                ot = ot_pend[qc]
                if mo % 2 == 0:
                    nc.scalar.activation(
                        ot[:, 0, :], pp, mybir.ActivationFunctionType.Copy)
                else:
                    nc.vector.tensor_copy(ot[:, 1, :], pp)
                    eng = {"gpsimd": nc.gpsimd, "sync": nc.sync,
                           "scalar": nc.scalar}.get(OUT_DMA, nc.gpsimd)
                    eng.dma_start(
                        out_v[:, mo - 1:mo + 1, qc * 512:(qc + 1) * 512],
                        ot_pend.pop(qc)[:])

            out_cp_flip = [0]

            def proj_out(qc, mp, pp, tail=False, force_act=False):
                ot = pout.tile([P, 2, 512], BF16, tag="ot", name="ot")
                if force_act:
                    nc.scalar.activation(
                        ot[:], pp[:], mybir.ActivationFunctionType.Copy)
                    eng = nc.sync if tail else {
                        "gpsimd": nc.gpsimd, "sync": nc.sync,
                        "scalar": nc.scalar}.get(OUT_DMA, nc.gpsimd)
                    eng.dma_start(
                        out_v[:, 2 * mp:2 * mp + 2,
                              qc * 512:(qc + 1) * 512], ot[:])
                    return
                if OUT_CP == "split":
                    # one half per engine: the PSUM pair frees after the
                    # faster of the two queues, not a 2-pair serial chain
                    nc.scalar.activation(
                        ot[:, 0, :], pp[:, 0, :],
                        mybir.ActivationFunctionType.Copy)
                    nc.vector.tensor_copy(ot[:, 1, :], pp[:, 1, :])
                else:
                    use_act = (OUT_CP == "act") or (
                        OUT_CP == "alt" and out_cp_flip[0] % 2 == 0)
                    out_cp_flip[0] += 1
                    if use_act:
                        nc.scalar.activation(
                            ot[:], pp[:], mybir.ActivationFunctionType.Copy)
                    else:
                        nc.vector.tensor_copy(ot[:], pp[:])
                eng = nc.sync if tail else {
                    "gpsimd": nc.gpsimd, "sync": nc.sync,
                    "scalar": nc.scalar}.get(OUT_DMA, nc.gpsimd)
                eng.dma_start(
                    out_v[:, 2 * mp:2 * mp + 2, qc * 512:(qc + 1) * 512],
                    ot[:])

            # ---- priming: main terms of k AND q first (they gate only on
            # the w8/x8 DMAs), residual terms after (their DMAs land later),
            # v0 on scratch between the copies
            kp = qk_pair_mm("k", 0)
            qp = qk_pair_mm("q", 0)
            qk_copy("k", 0, kp, 0)
            qk_copy("k", 0, kp, 1)
            vp0 = v_pair_mm(0)
            qk_copy("q", 0, qp, 0)
            qk_copy("q", 0, qp, 1)
            v_copy(0, vp0)

            # ---- block order ----
            block_list = []
            for qc in range(NQ):
                heads = (2, 3, 0, 1) if qc == NQ - 1 else (0, 1, 2, 3)
                for h in heads:
                    block_list.append((qc, h))
            pair_stream = [
                (qc, h, p) for (qc, h) in block_list for p in range(NP)]
            NSLOT = len(pair_stream)  # 128

            # ---- filler schedule (slot -> closures) ----
            fill = {g: [] for g in range(NSLOT)}

            def qk_filler(which, n):
                st = {}

                def mmA():
                    st["ps"] = qk_pair_mm(which, n)

                def cps():
                    pst = st.pop("ps")
                    qk_copy(which, n, pst, 0)
                    qk_copy(which, n, pst, 1)

                return mmA, cps

            def v_filler(sp):
                st = {}

                def mm():
                    st["ps"] = v_pair_mm(sp)

                def cp():
                    v_copy(sp, st.pop("ps"))

                return mm, cp

            k_ready_slot = {0: -1}
            q_ready_slot = {0: -1}

            # k pairs n1-3 and v pairs 1-7 JIT inside the first two blocks
            vmm = {sp: v_filler(sp) for sp in range(1, NP)}
            kmm = {n: qk_filler("k", n) for n in (1, 2, 3)}
            qmm = {n: qk_filler("q", n) for n in (1, 2, 3)}
            jit = [
                (0, [kmm[1][0]]),
                (1, [kmm[1][1], kmm[2][0]]),
                (2, [kmm[2][1], kmm[3][0]]),
                (3, [kmm[3][1], vmm[1][0]]),
                (4, [vmm[1][1], vmm[2][0], qmm[1][0]]),
                (5, [vmm[2][1], vmm[3][0], qmm[1][1]]),
                (6, [vmm[3][1], vmm[4][0]]),
                (7, [vmm[4][1], vmm[5][0]]),
                (8, [vmm[5][1], vmm[6][0]]),
                (9, [vmm[6][1], vmm[7][0]]),
                (10, [vmm[7][1]]),
                (12, [qmm[2][0]]),
                (13, [qmm[2][1]]),
                (20, [qmm[3][0]]),
                (21, [qmm[3][1]]),
            ]
            for g, fs in jit:
                fill[g].extend(fs)
            # "ready" = first slot whose SE advance may read the copies:
            # fillers run AFTER the SE advances within a slot, so +1
            k_ready_slot[1] = 2
            k_ready_slot[2] = 3
            k_ready_slot[3] = 4
            q_ready_slot[1] = 6
            q_ready_slot[2] = 14
            q_ready_slot[3] = 22


            # ---- main slot loop ----
            pending = []  # (due_slot, order, fn)
            slot_no = [0]
            order_no = [0]

            def at_slot(lag, fn):
                pending.append((slot_no[0] + lag, order_no[0], fn))
                order_no[0] += 1

            def run_due():
                pending.sort(key=lambda e: (e[0], e[1]))
                while pending and pending[0][0] <= slot_no[0]:
                    pending.pop(0)[2]()

            blk_idx = {bh: i for i, bh in enumerate(block_list)}
            tail_tp = []
            for qc in range(NQ):
                for pair in range(2):
                    last = max(blk_idx[(qc, 2 * pair)],
                               blk_idx[(qc, 2 * pair + 1)])
                    if last + 1 >= len(block_list):
                        tail_tp.append((qc, pair))

            se_idx = [0]
            av_emitted = [0]

            def se_gate_ok():
                if se_idx[0] >= NSLOT:
                    return False
                qc, h, p = pair_stream[se_idx[0]]
                g = slot_no[0]
                if k_ready_slot.get(p // 2, 10 ** 9) > g:
                    return False
                if q_ready_slot.get(qc, 10 ** 9) > g:
                    return False
                if se_idx[0] - av_emitted[0] >= PEXP_BUFS - 2:
                    return False
                return True

            def advance_se():
                qc, h, p = pair_stream[se_idx[0]]
                ps = score_pair_mm(h, qc, p)
                ex = emit_exp(ps, se_idx[0])
                j = se_idx[0]
                # AV trails; shorten the drain for the last block. Block 0
                # AVs wait until the v JIT vacates the scratch banks.
                lag = AV_LAG
                if j >= NSLOT - AV_LAG:
                    lag = max(1, NSLOT - 1 - j)
                # blocks 0/1: their po banks double as the v JIT scratch;
                # first AV (start=True bank zero) must trail the last v copy
                # emitted there (v6 -> scr0 @ slot 9, v7 -> scr1 @ slot 10),
                # and every AV must pop only after its v pair's copy was
                # EMITTED (run_due precedes fillers within a slot, so the
                # floor is copy_slot + 1)
                if j < NP:
                    p_ = j % NP
                    lag = max(lag, (11 if p_ == NP - 1 else 10) - slot_no[0])
                elif j < 2 * NP:
                    lag = max(lag, 11 - slot_no[0])
                at_slot(lag, lambda qc=qc, h=h, p=p, ex=ex: (
                    emit_av(h, qc, p, ex),
                    av_emitted.__setitem__(0, av_emitted[0] + 1)))
                if p == NP - 1:
                    at_slot(lag + EPI_LAG,
                            lambda h=h, qc=qc: epilogue(h, qc))
                    if qc >= 1 and not (qc == NQ - 1 and h < 2):
                        # proj singles reuse the epilogue's freed scratch
                        # bank. They MUST be emitted before AV(b+2, p0)
                        # (same bank parity) zeroes it: same due slot as the
                        # epilogue, consecutive order numbers. qc2's mo 0-3
                        # (qc3's last two windows) run in the tail instead.
                        scr = scr_sb[blk_idx[(qc, h)] % 2]
                        for moff in (0, 1):
                            mo = 2 * h + moff
                            at_slot(
                                lag + EPI_LAG,
                                lambda qc=qc, mo=mo, scr=scr:
                                    proj_out_single(
                                        qc - 1, mo,
                                        proj_single_mm(qc - 1, mo, scr)))
                se_idx[0] += 1

            for g in range(NSLOT):
                slot_no[0] = g
                if block_list[g // NP][0] not in ctxT_tiles and g % NP == 0:
                    qc = block_list[g // NP][0]
                    ctxT_tiles[qc] = pctxT.tile(
                        [P, 2, 512], BF16, tag="ctxT", name=f"ctxT{qc}")
                run_due()
                # own-pair score/exp first, then fillers, then extra lead
                n_emit = 0
                while (n_emit < SE_PER_SLOT and se_gate_ok()
                       and se_idx[0] <= g + 1 + SE_LEAD):
                    advance_se()
                    n_emit += 1
                for f in fill[g]:
                    f()
                if (se_idx[0] <= g and se_gate_ok()):
                    advance_se()

            # ---- tail ----
            slot_no[0] = NSLOT
            while se_idx[0] < NSLOT:
                slot_no[0] += 1
                run_due()
                if se_gate_ok():
                    advance_se()
                else:
                    break
            # the pre-drain loop empties pending; pop any stragglers
            pending.sort(key=lambda e: (e[0], e[1]))
            qc_last = NQ - 1
            while pending:
                pending.pop(0)[2]()
            # qc2's mo 0-3 as ring pairs: their copies and DMAs overlap the
            # last block's drain and the tail transposes. Ring WAR chain:
            # qc2mp0<-sc125, qc2mp1<-sc126, tpA<-sc127, tpB<-qc2mp0(copy),
            # mp0<-qc2mp1, mp1<-tpA, mp2<-tpB, mp3<-mp0(copy) -- all ordered.
            for mp in (0, 1):
                proj_out(qc_last - 1, mp, proj_pair_mm(qc_last - 1, mp),
                         force_act=True)
            for qcp, pair in tail_tp:
                # PE transposes, one qs per ring bank (bf16 views); the two
                # PSUM->SBUF copies split across ACT and DVE
                for qsh in (0, 1):
                    tpb = ring_pair().bitcast(BF16)  # [128, 2, 1024]
                    for half in (0, 1):
                        qs = 2 * qsh + half
                        nc.tensor.transpose(
                            tpb[:, half, 0:P],
                            ctx_tiles[pair][:, qs, :], ident_sb[:])
                    dst = ctxT_tiles[qcp][:, pair, qsh * 256:(qsh + 1) * 256]\
                        .rearrange("p (a c) -> p a c", a=2)
                    if qsh == 0:
                        nc.scalar.activation(
                            dst, tpb[:, :, 0:P],
                            mybir.ActivationFunctionType.Copy)
                    else:
                        nc.vector.tensor_copy(dst, tpb[:, :, 0:P])
            # tail out: per-half copies alternate engines right behind each
            # matmul pair so only the last ~650ns copy trails the PE stream
            ot_tail = [pout.tile([P, 2, 512], BF16, tag="ot4", bufs=4,
                                 name=f"ot4_{i}") for i in range(4)]
            for mp in range(4):
                pp = proj_pair_mm(qc_last, mp)
                ot = ot_tail[mp]
                if mp % 2 == 0:
                    nc.scalar.activation(
                        ot[:, 0, :], pp[:, 0, :],
                        mybir.ActivationFunctionType.Copy)
                    nc.vector.tensor_copy(ot[:, 1, :], pp[:, 1, :])
                else:
                    nc.vector.tensor_copy(ot[:, 0, :], pp[:, 0, :])
                    nc.scalar.activation(
                        ot[:, 1, :], pp[:, 1, :],
                        mybir.ActivationFunctionType.Copy)
                nc.sync.dma_start(
                    out_v[:, 2 * mp:2 * mp + 2,
                          qc_last * 512:(qc_last + 1) * 512],
                    ot[:])
            if DEBUG:
                nc.sync.dma_start(dq8_d[:], q8_sb[:])
                nc.sync.dma_start(dk8_d[:], k8_sb[:])
                nc.sync.dma_start(dv_d[:], v_all[:])
                nc.sync.dma_start(dctxT_d[:], ctxT_tiles[qc_last][:])

    nc.compile()
    return nc


_NC = None


def _get_program():
    global _NC
    if _NC is None:
        _NC = build_program()
    return _NC


def _perm256():
    """Row order for a 256-row (4-head) q or k block: two 128-chunks
    (hd-high 0/1), partition p = 32*h + dlo -> row h*64 + dhi*32 + dlo."""
    idx = []
    for dhi in range(2):
        for h in range(4):
            for dlo in range(32):
                idx.append(h * 64 + dhi * 32 + dlo)
    return np.array(idx)


def _pack_dr(a):
    """[1024 contraction, M] -> [128 p, 4 kc, 2 j, M] with c = kc*256+2p+j."""
    m = a.shape[1]
    return np.ascontiguousarray(
        a.reshape(KC, P, 2, m).transpose(1, 0, 2, 3))


def prepare_inputs(x, w_qkv, b_qkv, w_proj):
    """Build the 8 per-core input maps from full inputs."""
    perm = _perm256()
    f8 = ml_dtypes.float8_e4m3
    f85 = ml_dtypes.float8_e5m2
    in_maps = []
    for c in range(8):
        b, hg = c // 4, c % 4
        sl = slice(hg * 256, (hg + 1) * 256)
        w_q, w_k = w_qkv[0:D][sl], w_qkv[D:2 * D][sl]
        w_v = w_qkv[2 * D:][sl]
        b_q, b_k = b_qkv[0:D][sl], b_qkv[D:2 * D][sl]
        wqk = np.vstack([w_q[perm], w_k[perm]]).T  # [1024 c, 512 m]
        bqk = np.concatenate([b_q[perm], b_k[perm]])
        wv = w_v.T  # [1024, 256]
        xt = np.ascontiguousarray(x[b].T)  # [1024, 2048]

        x8 = xt.astype(f8)
        xr5 = (xt - x8.astype(np.float32)).astype(f85)
        w8 = wqk.astype(f8)
        wr5 = (wqk - w8.astype(np.float32)).astype(f85)
        wv8 = wv.astype(f8)
        wvr5 = (wv - wv8.astype(np.float32)).astype(f85)

        in_maps.append(
            {
                "x8": _pack_dr(x8),
                "xr5": _pack_dr(xr5),
                "w8": _pack_dr(w8),
                "wr5": _pack_dr(wr5),
                "wv8": _pack_dr(wv8),
                "wvr5": _pack_dr(wvr5),
                "bqk": np.ascontiguousarray(
                    bqk.reshape(4, P).T.astype(np.float32)),
                "bv": np.ascontiguousarray(
                    b_qkv[2 * D:][sl].astype(ml_dtypes.bfloat16)),
                "wp": np.ascontiguousarray(
                    w_proj[:, sl].T).astype(ml_dtypes.bfloat16),
            }
        )
    return in_maps


def run(in_maps, **kwargs):
    nc = _get_program()
    last_err = None
    for _ in range(3):
        try:
            res = run_bass_kernel_spmd(
                nc, in_maps, core_ids=list(range(8)), **kwargs)
            res.results = [
                {k: np.array(v, dtype=np.float32) for k, v in r.items()}
                for r in res.results
            ]
            return res
        except Exception as e:  # transient NRT_EXEC_UNIT_UNRECOVERABLE etc.
            last_err = e
    raise last_err


def assemble(results, b_proj):
    out = np.empty((B, S, D), dtype=np.float32)
    for b in range(B):
        acc = results[4 * b]["out"].copy()
        for hg in range(1, 4):
            acc += results[4 * b + hg]["out"]
        out[b] = acc.T + b_proj
    return out


def kernel(x, w_qkv, b_qkv, w_proj, b_proj):
    x = np.asarray(x, dtype=np.float32)
    w_qkv = np.asarray(w_qkv, dtype=np.float32)
    b_qkv = np.asarray(b_qkv, dtype=np.float32)
    w_proj = np.asarray(w_proj, dtype=np.float32)
    b_proj = np.asarray(b_proj, dtype=np.float32)
    res = run(prepare_inputs(x, w_qkv, b_qkv, w_proj))
    return assemble(res.results, b_proj)
